# revision 30
# baseline (speedup 1.0000x reference)
"""Trainium2 Bass kernel for dynamic low-pass filter decomposition.

Module: global-avg-pool -> 1x1 conv -> BN -> softmax over 3x3 taps gives a
per-(sample, group) 3x3 kernel; applied as a reflect-padded depthwise conv
over x; returns (low, x - low).

Sharding: data-parallel over batch n=8 across 8 NeuronCores (1 sample/core).

All HBM traffic is fp16.  x is pre-padded on the host into [128, 98*196]
per core (partition p = c*2 + h; 96 image rows + 2 reflected halo rows per
half; each row stored [., padL, 192 cols, padR, .] with the w-reflect values
in the pads) so loads are pure flat DMA and every 3x3 tap on device is a
plain shifted flat view with NO edge fixups.  low/high keep the same padded
row stride in SBUF/DRAM (pad lanes compute garbage, host strips them), so
the whole main loop is flat 512-col chunks:

  PE     7 taps as diagonal fp16 matmuls into PSUM (per-mm LDWEIGHTS is
         fully hidden behind the previous matmul: measured 164ns cadence)
  ScalarE PSUM->SBUF copy (fp32 -> fp16)
  VectorE 2 even-shift taps as tensor_scalar_mul (4x) + tensor_tensor add
         (2x)  [scalar_tensor_tensor only has a 1x uop: measured], then
         high = x - low (2x)

Weight generation runs on-device from per-load-tile partial sums; warmup
matmuls keep the PE HAM clock at 2.4 GHz through the load phase.
"""
import sys

sys.path.insert(0, "/opt/trn_rl_repo")

import numpy as np
from contextlib import ExitStack

import concourse.bass as bass
import concourse.tile as tile
from concourse import bacc, mybir
from concourse.bass_utils import run_bass_kernel_spmd

dt = mybir.dt
f32 = dt.float32
f16 = dt.float16

KS = 3
GROUP = 8
IC = 64
BN_EPS = 1e-5
N = 8
H = W = 192
RH = 96                  # rows per half-image
RS = 196                 # row stride (2 pad cols + 192 + 2 pad cols)
NROWS = 98               # 96 image rows + top/bottom halo
XPLEN = NROWS * RS       # padded x elems per partition (19208)
XB = 4 + RS              # xt idx of out elem 0 (skip front slack + halo row)
OLEN = RH * RS           # padded out elems per partition (18816)
CH = 512                 # cols per PSUM chunk
NCHUNK = (OLEN + CH - 1) // CH          # 37 (last chunk 384)
GROUPS = [6, 6, 6, 6, 6, 6, 1]          # chunks per compute group

# load tiles (rows each, descending so the last partial-sum lands early);
# partial sums round-robin DVE/ACT/GPSIMD so they trail each load closely
LD_ROWS = [12, 12, 12, 12, 12, 12, 12, 8, 4]
LD_ENG = ["V", "A", "V", "A", "V", "A", "V", "A", "V"]

PE_TAPS = [0, 2, 3, 4, 5, 6, 8]   # diagonal fp16 matmuls
DVE_TAPS = [1, 7]                 # even shift: ts_mul(4x) + tt_add(2x)
WARM_PER_ST = 6                   # HAM warmup matmuls issued per load tile


def _shift(k):
    di, dj = k // 3, k % 3
    return (di - 1) * RS + (dj - 1)


def _build_program():
    """Trace the SPMD Bass program (same for every core)."""
    nc = bacc.Bacc("TRN2", target_bir_lowering=False, debug=False,
                   num_devices=N)

    x_d = nc.dram_tensor("x", [128, XPLEN], f16, kind="ExternalInput")
    at_d = nc.dram_tensor("at128", [128, 72], f32, kind="ExternalInput")
    b_d = nc.dram_tensor("b72", [72, 1], f32, kind="ExternalInput")
    r9_d = nc.dram_tensor("r9", [72, 9], f32, kind="ExternalInput")
    g_d = nc.dram_tensor("g728", [72, 8], f32, kind="ExternalInput")
    h_d = nc.dram_tensor("h8128", [8, 128], f32, kind="ExternalInput")
    eye_d = nc.dram_tensor("eye", [128, 128], f16, kind="ExternalInput")
    low_d = nc.dram_tensor("low", [128, OLEN], f16, kind="ExternalOutput")
    high_d = nc.dram_tensor("high", [128, OLEN], f16, kind="ExternalOutput")

    def dram_flat(tensor, base, inner, pitch):
        """Flat (128, inner) DRAM AP: full 16-engine DMA spray."""
        return bass.AP(tensor, base, [[pitch, 128], [1, inner]])

    with tile.TileContext(nc) as tc, ExitStack() as ctx:
        cpool = ctx.enter_context(tc.tile_pool(name="consts", bufs=1))
        xpool = ctx.enter_context(tc.tile_pool(name="x", bufs=1))
        wpool = ctx.enter_context(tc.tile_pool(name="w", bufs=1))
        opool = ctx.enter_context(tc.tile_pool(name="out", bufs=1))
        tpool = ctx.enter_context(tc.tile_pool(name="tmp", bufs=2))

        xt = xpool.tile([128, 4 + XPLEN + 4], f16)
        low = opool.tile([128, OLEN], f16)
        high = opool.tile([128, OLEN], f16)

        # ---- consts first on sync queue (tiny; eye feeds PE warmup) ----
        at_s = cpool.tile([128, 72], f32)
        b_s = cpool.tile([72, 1], f32)
        r9_s = cpool.tile([72, 9], f32)
        g_s = cpool.tile([72, 8], f32)
        h_s = cpool.tile([8, 128], f32)
        eye_s = cpool.tile([128, 128], f16)
        for t, d in ((eye_s, eye_d), (at_s, at_d), (b_s, b_d), (r9_s, r9_d),
                     (g_s, g_d), (h_s, h_d)):
            nc.sync.dma_start(t[:], d.ap())

        # ---- x loads: flat, spread over 4 DMA queues ----
        ld_q = [nc.sync, nc.scalar, nc.gpsimd]
        nld = len(LD_ROWS)
        ld_r0 = [int(v) for v in np.cumsum([0] + LD_ROWS)[:-1]]
        for s, (r0, rows) in enumerate(zip(ld_r0, LD_ROWS)):
            a = (r0 + 1) * RS
            ld_q[s % 3].dma_start(
                xt[:, 4 + a:4 + a + rows * RS],
                dram_flat(x_d.ap().tensor, a, rows * RS, XPLEN))
        # halo rows (host-prepped reflections)
        nc.sync.dma_start(xt[:, 4:4 + RS],
                          dram_flat(x_d.ap().tensor, 0, RS, XPLEN))
        nc.scalar.dma_start(xt[:, 4 + 97 * RS:4 + 98 * RS],
                            dram_flat(x_d.ap().tensor, 97 * RS, RS, XPLEN))

        def row_view(row0, nrows, extra=0):
            """[[RS, nrows], [1, W]] image-cols view from buffer row row0."""
            s0 = 4 + row0 * RS + 2 + extra
            return xt[:, s0:s0 + nrows * RS].rearrange(
                "p (r w) -> p r w", w=RS)[:, :, 0:W]

        # ---- partial sums for the global mean, one per load tile ----
        ndv = sum(r for s, r in enumerate(LD_ROWS) if LD_ENG[s] == "V")
        nda = sum(1 for e in LD_ENG if e == "A")
        rowsum_v = wpool.tile([128, ndv], f32)
        partials_a = wpool.tile([128, nda], f32)
        rscratch = wpool.tile([128, 3072], f16)
        dv0 = da0 = 0
        for s, (r0, rows) in enumerate(zip(ld_r0, LD_ROWS)):
            src = row_view(r0 + 1, rows)
            if LD_ENG[s] == "V":
                nc.vector.tensor_reduce(rowsum_v[:, dv0:dv0 + rows], src,
                                        axis=mybir.AxisListType.X,
                                        op=mybir.AluOpType.add)
                dv0 += rows
            else:
                rsv = rscratch[:, :rows * W].rearrange(
                    "p (r w) -> p r w", w=W)
                nc.scalar.activation(rsv, src,
                                     mybir.ActivationFunctionType.Copy,
                                     accum_out=partials_a[:, da0:da0 + 1])
                da0 += 1

        # ---- HAM warmup: keep PE busy through the load phase ----
        with tc.tile_pool(name="wpsum", bufs=1,
                          space=bass.MemorySpace.PSUM) as wpsum:
            warm = wpsum.tile([128, 512], f32, tag="warm")
            for s, (r0, rows) in enumerate(zip(ld_r0, LD_ROWS)):
                a = 4 + (r0 + 1) * RS
                for i in range(WARM_PER_ST):
                    nc.tensor.matmul(warm[:], eye_s[:], xt[:, a:a + 512],
                                     start=True, stop=True)

            # ---- weight generation (all fp32, as the math demands) ----
            sum_v = wpool.tile([128, 1], f32)
            nc.vector.tensor_reduce(sum_v[:], rowsum_v[:],
                                    axis=mybir.AxisListType.X,
                                    op=mybir.AluOpType.add)
            sum_a = wpool.tile([128, 1], f32)
            nc.vector.tensor_reduce(sum_a[:], partials_a[:],
                                    axis=mybir.AxisListType.X,
                                    op=mybir.AluOpType.add)
            sum128 = wpool.tile([128, 1], f32)
            nc.vector.tensor_add(sum128[:], sum_v[:], sum_a[:])

            lf_p = wpsum.tile([72, 1], f32, tag="lf")
            nc.tensor.matmul(lf_p[:], at_s[:], sum128[:])
            e72 = wpool.tile([72, 1], f32)
            nc.scalar.activation(e72[:], lf_p[:],
                                 mybir.ActivationFunctionType.Exp,
                                 bias=b_s[:, 0:1], scale=1.0)
            rhsw = wpool.tile([72, 9], f32)
            nc.vector.tensor_scalar_mul(rhsw[:], r9_s[:], e72[:, 0:1])
            w89_p = wpsum.tile([8, 9], f32, tag="w89")
            nc.tensor.matmul(w89_p[:], g_s[:], rhsw[:])
            s8 = wpool.tile([8, 1], f32)
            nc.vector.tensor_reduce(s8[:], w89_p[:],
                                    axis=mybir.AxisListType.X,
                                    op=mybir.AluOpType.add)
            r8 = wpool.tile([8, 1], f32)
            nc.vector.reciprocal(r8[:], s8[:])
            w89s = wpool.tile([8, 9], f32)
            nc.vector.tensor_scalar_mul(w89s[:], w89_p[:], r8[:, 0:1])
            wbig_p = wpsum.tile([128, 9], f32, tag="wbig")
            nc.tensor.matmul(wbig_p[:], h_s[:], w89s[:])
            w128 = wpool.tile([128, 9], f32)
            nc.scalar.copy(w128[:], wbig_p[:])

            # diagonal fp16 weight matrices for the PE taps (scalars read
            # straight from PSUM so PE needn't wait for the w128 copy)
            diag = {}
            for k in PE_TAPS:
                diag[k] = wpool.tile([128, 128], f16, name=f"diag{k}")
            for k in PE_TAPS:
                nc.vector.tensor_scalar_mul(diag[k][:], eye_s[:],
                                            wbig_p[:, k:k + 1])

        # ---- main loop: flat 512-col chunks over the padded out layout ----
        with tc.tile_pool(name="psum", bufs=8,
                          space=bass.MemorySpace.PSUM) as psum:
            c0 = 0
            for s, nch in enumerate(GROUPS):
                g0 = c0 * CH
                glen = min(OLEN, (c0 + nch) * CH) - g0
                acc = []
                for i in range(nch):
                    cl = min(CH, OLEN - (c0 + i) * CH)
                    acc.append(psum.tile([128, cl], f32, tag="acc",
                                         name=f"acc{s}_{i}",
                                         padded_shape=[128, 512]))
                taps = PE_TAPS if s % 2 == 0 else PE_TAPS[::-1]
                for k in taps:
                    for i in range(nch):
                        a = XB + (c0 + i) * CH + _shift(k)
                        nc.tensor.matmul(acc[i][:], diag[k][:],
                                         xt[:, a:a + acc[i].shape[1]],
                                         start=(k == taps[0]),
                                         stop=(k == taps[-1]))
                for i in range(nch):
                    o = (c0 + i) * CH
                    nc.scalar.copy(low[:, o:o + acc[i].shape[1]], acc[i][:])
                for k in DVE_TAPS:
                    # scalar_tensor_tensor only has a 1x uop; ts_mul (4x) +
                    # tt add (2x) is faster for fp16 despite two passes
                    tmp = tpool.tile([128, 3072], f16, tag="tmp")
                    a = XB + g0 + _shift(k)
                    nc.vector.tensor_scalar_mul(tmp[:, :glen],
                                                xt[:, a:a + glen],
                                                w128[:, k:k + 1])
                    nc.vector.tensor_tensor(low[:, g0:g0 + glen],
                                            low[:, g0:g0 + glen],
                                            tmp[:, :glen],
                                            op=mybir.AluOpType.add)
                nc.vector.tensor_tensor(high[:, g0:g0 + glen],
                                        xt[:, XB + g0:XB + g0 + glen],
                                        low[:, g0:g0 + glen],
                                        op=mybir.AluOpType.subtract)
                nc.scalar.dma_start(
                    dram_flat(low_d.ap().tensor, g0, glen, OLEN),
                    low[:, g0:g0 + glen])
                nc.sync.dma_start(
                    dram_flat(high_d.ap().tensor, g0, glen, OLEN),
                    high[:, g0:g0 + glen])
                c0 += nch

    nc.compile()
    return nc


_nc_cache = None


def _get_program():
    global _nc_cache
    if _nc_cache is None:
        # NOTE: ldw-opt stays OFF (walrus rejects 16-bit LDWEIGHTS with
        # --enable-ldw-opt=true); per-mm LDWEIGHTS is fully hidden behind
        # the previous matmul (measured 164ns cadence for 384-col mms).
        _nc_cache = _build_program()
    return _nc_cache


def _host_consts(conv_w, bn_gamma, bn_beta, bn_mean, bn_var):
    s_a = bn_gamma / np.sqrt(bn_var + BN_EPS)
    b72 = (bn_beta - bn_mean * s_a).astype(np.float32).reshape(72, 1)
    A = (conv_w * s_a[:, None]) / np.float32(H * W)
    p = np.arange(128)
    at128 = np.ascontiguousarray(A.T[p // 2]).astype(np.float32)  # (128, 72)
    oc = np.arange(72)
    r9 = (oc[:, None] % 9 == np.arange(9)[None, :]).astype(np.float32)
    g728 = (oc[:, None] // 9 == np.arange(8)[None, :]).astype(np.float32)
    h8128 = (np.arange(8)[:, None] == (p[None, :] // 16)).astype(np.float32)
    eye = np.eye(128, dtype=np.float16)
    return dict(at128=at128, b72=b72, r9=r9, g728=g728, h8128=h8128, eye=eye)


def _pad_x(x16):
    """(n, 64, 192, 192) fp16 -> (n, 128, 98*196): 96 rows split into two
    halves stacked in the partition dim, one reflected halo row above and
    below each half, and each row stored as [., padL, 192 cols, padR, .]
    so 3x3 taps on device are plain shifted flat views."""
    n = x16.shape[0]
    xp = np.zeros((n, 64, 2, NROWS, RS), dtype=np.float16)
    xp[:, :, :, 1:97, 2:194] = x16.reshape(n, 64, 2, RH, W)
    xp[:, :, 0, 0, 2:194] = x16[:, :, 1]        # reflect of row -1
    xp[:, :, 1, 0, 2:194] = x16[:, :, 95]       # halo above bottom half
    xp[:, :, 0, 97, 2:194] = x16[:, :, 96]      # halo below top half
    xp[:, :, 1, 97, 2:194] = x16[:, :, 190]     # reflect of row 192
    xp[..., 1] = xp[..., 3]                     # reflect of col -1
    xp[..., 194] = xp[..., 192]                 # reflect of col 192
    return np.ascontiguousarray(xp.reshape(n, 128, XPLEN))


def _prepare(x, conv_w, bn_gamma, bn_beta, bn_mean, bn_var):
    x16 = np.asarray(x, dtype=np.float16)
    xp = _pad_x(x16)
    consts = _host_consts(np.asarray(conv_w, np.float32),
                          np.asarray(bn_gamma, np.float32),
                          np.asarray(bn_beta, np.float32),
                          np.asarray(bn_mean, np.float32),
                          np.asarray(bn_var, np.float32))
    return [dict(x=xp[i], **consts) for i in range(N)]


def _unpad(a):
    """[128, 96*196] padded-rows -> (64, 192, 192) fp32."""
    return a.reshape(64, 2, RH, RS)[..., 2:194].reshape(
        64, H, W).astype(np.float32)


def _collect(res):
    low = np.stack([_unpad(res[i]["low"]) for i in range(N)])
    high = np.stack([_unpad(res[i]["high"]) for i in range(N)])
    return low, high


def kernel(x, conv_w, bn_gamma, bn_beta, bn_mean, bn_var):
    in_maps = _prepare(x, conv_w, bn_gamma, bn_beta, bn_mean, bn_var)
    nc = _get_program()
    res = run_bass_kernel_spmd(nc, in_maps, list(range(N))).results
    return _collect(res)


if __name__ == "__main__":
    rng = np.random.default_rng(0)
    demo = dict(
        x=rng.standard_normal((N, IC, H, W), dtype=np.float32),
        conv_w=rng.standard_normal((72, 64)).astype(np.float32),
        bn_gamma=np.ones(72, np.float32),
        bn_beta=np.zeros(72, np.float32),
        bn_mean=rng.standard_normal(72).astype(np.float32) * 0.1,
        bn_var=rng.uniform(0.5, 1.5, 72).astype(np.float32),
    )
    low, high = kernel(**demo)
    print("ok", low.shape, high.shape)


# revision 34
# speedup vs baseline: 1.0552x; 1.0552x over previous
"""Trainium2 Bass kernel for dynamic low-pass filter decomposition.

Module: global-avg-pool -> 1x1 conv -> BN -> softmax over 3x3 taps gives a
per-(sample, group) 3x3 kernel; applied as a reflect-padded depthwise conv
over x; returns (low, x - low).

Sharding: data-parallel over batch n=8 across 8 NeuronCores (1 sample/core).

All HBM traffic is fp16.  x is pre-padded on the host into [128, 98*196]
per core (partition p = c*2 + h; 96 image rows + 2 reflected halo rows per
half; each row stored [., padL, 192 cols, padR, .] with the w-reflect values
in the pads) so loads are pure flat DMA and every 3x3 tap on device is a
plain shifted flat view with NO edge fixups.  low/high keep the same padded
row stride in SBUF/DRAM (pad lanes compute garbage, host strips them), so
the whole main loop is flat 512-col chunks:

  PE     7 taps as diagonal fp16 matmuls into PSUM (per-mm LDWEIGHTS is
         fully hidden behind the previous matmul: measured 164ns cadence)
  ScalarE PSUM->SBUF copy (fp32 -> fp16)
  VectorE 2 even-shift taps as tensor_scalar_mul (4x) + tensor_tensor add
         (2x)  [scalar_tensor_tensor only has a 1x uop: measured], then
         high = x - low (2x)

Weight generation runs on-device from per-load-tile partial sums; warmup
matmuls keep the PE HAM clock at 2.4 GHz through the load phase.
"""
import sys

sys.path.insert(0, "/opt/trn_rl_repo")

import numpy as np
from contextlib import ExitStack

import concourse.bass as bass
import concourse.tile as tile
from concourse import bacc, mybir
from concourse.bass_utils import run_bass_kernel_spmd

dt = mybir.dt
f32 = dt.float32
f16 = dt.float16

KS = 3
GROUP = 8
IC = 64
BN_EPS = 1e-5
N = 8
H = W = 192
RH = 96                  # rows per half-image
RS = 196                 # row stride (2 pad cols + 192 + 2 pad cols)
NROWS = 98               # 96 image rows + top/bottom halo
XPLEN = NROWS * RS       # padded x elems per partition (19208)
XB = 4 + RS              # xt idx of out elem 0 (skip front slack + halo row)
OLEN = RH * RS           # padded out elems per partition (18816)
CH = 512                 # cols per PSUM chunk
NCHUNK = (OLEN + CH - 1) // CH          # 37 (last chunk 384)
GROUPS = [6, 6, 6, 6, 6, 4, 2, 1]       # chunks per group (tapered tail)

# load tiles (rows each, descending so the last partial-sum lands early);
# partial sums round-robin DVE/ACT/GPSIMD so they trail each load closely
# queue s%3 gets [16, 12, 4] rows: all queues drain together and the three
# tiny trailing tiles land (and reduce) last with ~1us of work left
LD_ROWS = [16, 16, 16, 12, 12, 12, 4, 4, 4]
LD_ENG = ["V", "A", "V", "A", "V", "A", "V", "A", "V"]

PE_TAPS = [0, 2, 3, 4, 5, 6, 8]   # diagonal fp16 matmuls
DVE_TAPS = [1, 7]                 # even shift: ts_mul(4x) + tt_add(2x)
WARM_PER_ST = 4                   # HAM warmup matmuls per early load tile
WARM_TILES = 6                    # only early tiles: warmups must drain
                                  # before the weight-chain matmuls queue


def _shift(k):
    di, dj = k // 3, k % 3
    return (di - 1) * RS + (dj - 1)


def _build_program():
    """Trace the SPMD Bass program (same for every core)."""
    nc = bacc.Bacc("TRN2", target_bir_lowering=False, debug=False,
                   num_devices=N)

    x_d = nc.dram_tensor("x", [128, XPLEN], f16, kind="ExternalInput")
    at_d = nc.dram_tensor("at128", [128, 72], f32, kind="ExternalInput")
    b_d = nc.dram_tensor("b72", [72, 1], f32, kind="ExternalInput")
    r9_d = nc.dram_tensor("r9", [72, 9], f32, kind="ExternalInput")
    g_d = nc.dram_tensor("g728", [72, 8], f32, kind="ExternalInput")
    h_d = nc.dram_tensor("h8128", [8, 128], f32, kind="ExternalInput")
    eye_d = nc.dram_tensor("eye", [128, 128], f16, kind="ExternalInput")
    low_d = nc.dram_tensor("low", [128, OLEN], f16, kind="ExternalOutput")
    high_d = nc.dram_tensor("high", [128, OLEN], f16, kind="ExternalOutput")

    def dram_flat(tensor, base, inner, pitch):
        """Flat (128, inner) DRAM AP: full 16-engine DMA spray."""
        return bass.AP(tensor, base, [[pitch, 128], [1, inner]])

    with tile.TileContext(nc) as tc, ExitStack() as ctx:
        cpool = ctx.enter_context(tc.tile_pool(name="consts", bufs=1))
        xpool = ctx.enter_context(tc.tile_pool(name="x", bufs=1))
        wpool = ctx.enter_context(tc.tile_pool(name="w", bufs=1))
        opool = ctx.enter_context(tc.tile_pool(name="out", bufs=1))
        tpool = ctx.enter_context(tc.tile_pool(name="tmp", bufs=2))

        xt = xpool.tile([128, 4 + XPLEN + 4], f16)
        low = opool.tile([128, OLEN], f16)
        high = opool.tile([128, OLEN], f16)

        # ---- consts first on sync queue (tiny; eye feeds PE warmup) ----
        at_s = cpool.tile([128, 72], f32)
        b_s = cpool.tile([72, 1], f32)
        r9_s = cpool.tile([72, 9], f32)
        g_s = cpool.tile([72, 8], f32)
        h_s = cpool.tile([8, 128], f32)
        eye_s = cpool.tile([128, 128], f16)
        for t, d in ((eye_s, eye_d), (at_s, at_d), (b_s, b_d), (r9_s, r9_d),
                     (g_s, g_d), (h_s, h_d)):
            nc.sync.dma_start(t[:], d.ap())

        # ---- x loads: flat, spread over 4 DMA queues ----
        ld_q = [nc.sync, nc.scalar, nc.gpsimd]
        nld = len(LD_ROWS)
        ld_r0 = [int(v) for v in np.cumsum([0] + LD_ROWS)[:-1]]
        for s, (r0, rows) in enumerate(zip(ld_r0, LD_ROWS)):
            a = (r0 + 1) * RS
            ld_q[s % 3].dma_start(
                xt[:, 4 + a:4 + a + rows * RS],
                dram_flat(x_d.ap().tensor, a, rows * RS, XPLEN))
        # halo rows (host-prepped reflections)
        nc.sync.dma_start(xt[:, 4:4 + RS],
                          dram_flat(x_d.ap().tensor, 0, RS, XPLEN))
        nc.scalar.dma_start(xt[:, 4 + 97 * RS:4 + 98 * RS],
                            dram_flat(x_d.ap().tensor, 97 * RS, RS, XPLEN))

        def row_view(row0, nrows, extra=0):
            """[[RS, nrows], [1, W]] image-cols view from buffer row row0."""
            s0 = 4 + row0 * RS + 2 + extra
            return xt[:, s0:s0 + nrows * RS].rearrange(
                "p (r w) -> p r w", w=RS)[:, :, 0:W]

        # ---- partial sums for the global mean, one per load tile ----
        ndv = sum(r for s, r in enumerate(LD_ROWS) if LD_ENG[s] == "V")
        nda = sum(1 for e in LD_ENG if e == "A")
        rowsum_v = wpool.tile([128, ndv], f32)
        partials_a = wpool.tile([128, nda], f32)
        rscratch = wpool.tile([128, 3072], f16)
        dv0 = da0 = 0
        for s, (r0, rows) in enumerate(zip(ld_r0, LD_ROWS)):
            src = row_view(r0 + 1, rows)
            if LD_ENG[s] == "V":
                nc.vector.tensor_reduce(rowsum_v[:, dv0:dv0 + rows], src,
                                        axis=mybir.AxisListType.X,
                                        op=mybir.AluOpType.add)
                dv0 += rows
            else:
                rsv = rscratch[:, :rows * W].rearrange(
                    "p (r w) -> p r w", w=W)
                nc.scalar.activation(rsv, src,
                                     mybir.ActivationFunctionType.Copy,
                                     accum_out=partials_a[:, da0:da0 + 1])
                da0 += 1

        # ---- HAM warmup: keep PE busy through the load phase ----
        with tc.tile_pool(name="wpsum", bufs=1,
                          space=bass.MemorySpace.PSUM) as wpsum:
            warm = wpsum.tile([128, 512], f32, tag="warm")
            for s, (r0, rows) in enumerate(zip(ld_r0, LD_ROWS)):
                if s >= WARM_TILES:
                    continue
                a = 4 + (r0 + 1) * RS
                for i in range(WARM_PER_ST):
                    nc.tensor.matmul(warm[:], eye_s[:], xt[:, a:a + 512],
                                     start=True, stop=True)

            # ---- weight generation (all fp32, as the math demands) ----
            sum_v = wpool.tile([128, 1], f32)
            nc.vector.tensor_reduce(sum_v[:], rowsum_v[:],
                                    axis=mybir.AxisListType.X,
                                    op=mybir.AluOpType.add)
            sum_a = wpool.tile([128, 1], f32)
            nc.vector.tensor_reduce(sum_a[:], partials_a[:],
                                    axis=mybir.AxisListType.X,
                                    op=mybir.AluOpType.add)
            sum128 = wpool.tile([128, 1], f32)
            nc.vector.tensor_add(sum128[:], sum_v[:], sum_a[:])

            lf_p = wpsum.tile([72, 1], f32, tag="lf")
            nc.tensor.matmul(lf_p[:], at_s[:], sum128[:])
            e72 = wpool.tile([72, 1], f32)
            nc.scalar.activation(e72[:], lf_p[:],
                                 mybir.ActivationFunctionType.Exp,
                                 bias=b_s[:, 0:1], scale=1.0)
            rhsw = wpool.tile([72, 9], f32)
            nc.vector.tensor_scalar_mul(rhsw[:], r9_s[:], e72[:, 0:1])
            w89_p = wpsum.tile([8, 9], f32, tag="w89")
            nc.tensor.matmul(w89_p[:], g_s[:], rhsw[:])
            s8 = wpool.tile([8, 1], f32)
            nc.vector.tensor_reduce(s8[:], w89_p[:],
                                    axis=mybir.AxisListType.X,
                                    op=mybir.AluOpType.add)
            r8 = wpool.tile([8, 1], f32)
            nc.vector.reciprocal(r8[:], s8[:])
            w89s = wpool.tile([8, 9], f32)
            nc.vector.tensor_scalar_mul(w89s[:], w89_p[:], r8[:, 0:1])
            wbig_p = wpsum.tile([128, 9], f32, tag="wbig")
            nc.tensor.matmul(wbig_p[:], h_s[:], w89s[:])
            w128 = wpool.tile([128, 9], f32)
            nc.scalar.copy(w128[:], wbig_p[:])

            # diagonal fp16 weight matrices for the PE taps (scalars read
            # straight from PSUM so PE needn't wait for the w128 copy)
            diag = {}
            for k in PE_TAPS:
                diag[k] = wpool.tile([128, 128], f16, name=f"diag{k}")
            for k in PE_TAPS:
                nc.vector.tensor_scalar_mul(diag[k][:], eye_s[:],
                                            wbig_p[:, k:k + 1])

        # ---- main loop: flat 512-col chunks over the padded out layout ----
        with tc.tile_pool(name="psum", bufs=8,
                          space=bass.MemorySpace.PSUM) as psum:
            c0 = 0
            for s, nch in enumerate(GROUPS):
                g0 = c0 * CH
                glen = min(OLEN, (c0 + nch) * CH) - g0
                acc = []
                for i in range(nch):
                    cl = min(CH, OLEN - (c0 + i) * CH)
                    acc.append(psum.tile([128, cl], f32, tag="acc",
                                         name=f"acc{s}_{i}",
                                         padded_shape=[128, 512]))
                taps = PE_TAPS if s % 2 == 0 else PE_TAPS[::-1]
                for k in taps:
                    for i in range(nch):
                        a = XB + (c0 + i) * CH + _shift(k)
                        nc.tensor.matmul(acc[i][:], diag[k][:],
                                         xt[:, a:a + acc[i].shape[1]],
                                         start=(k == taps[0]),
                                         stop=(k == taps[-1]))
                for i in range(nch):
                    o = (c0 + i) * CH
                    nc.scalar.copy(low[:, o:o + acc[i].shape[1]], acc[i][:])
                for k in DVE_TAPS:
                    # scalar_tensor_tensor only has a 1x uop; ts_mul (4x) +
                    # tt add (2x) is faster for fp16 despite two passes
                    tmp = tpool.tile([128, 3072], f16, tag="tmp")
                    a = XB + g0 + _shift(k)
                    nc.vector.tensor_scalar_mul(tmp[:, :glen],
                                                xt[:, a:a + glen],
                                                w128[:, k:k + 1])
                    nc.vector.tensor_tensor(low[:, g0:g0 + glen],
                                            low[:, g0:g0 + glen],
                                            tmp[:, :glen],
                                            op=mybir.AluOpType.add)
                nc.vector.tensor_tensor(high[:, g0:g0 + glen],
                                        xt[:, XB + g0:XB + g0 + glen],
                                        low[:, g0:g0 + glen],
                                        op=mybir.AluOpType.subtract)
                nc.scalar.dma_start(
                    dram_flat(low_d.ap().tensor, g0, glen, OLEN),
                    low[:, g0:g0 + glen])
                nc.sync.dma_start(
                    dram_flat(high_d.ap().tensor, g0, glen, OLEN),
                    high[:, g0:g0 + glen])
                c0 += nch

    nc.compile()
    return nc


_nc_cache = None


def _get_program():
    global _nc_cache
    if _nc_cache is None:
        # NOTE: ldw-opt stays OFF (walrus rejects 16-bit LDWEIGHTS with
        # --enable-ldw-opt=true); per-mm LDWEIGHTS is fully hidden behind
        # the previous matmul (measured 164ns cadence for 384-col mms).
        _nc_cache = _build_program()
    return _nc_cache


def _host_consts(conv_w, bn_gamma, bn_beta, bn_mean, bn_var):
    s_a = bn_gamma / np.sqrt(bn_var + BN_EPS)
    b72 = (bn_beta - bn_mean * s_a).astype(np.float32).reshape(72, 1)
    A = (conv_w * s_a[:, None]) / np.float32(H * W)
    p = np.arange(128)
    at128 = np.ascontiguousarray(A.T[p // 2]).astype(np.float32)  # (128, 72)
    oc = np.arange(72)
    r9 = (oc[:, None] % 9 == np.arange(9)[None, :]).astype(np.float32)
    g728 = (oc[:, None] // 9 == np.arange(8)[None, :]).astype(np.float32)
    h8128 = (np.arange(8)[:, None] == (p[None, :] // 16)).astype(np.float32)
    eye = np.eye(128, dtype=np.float16)
    return dict(at128=at128, b72=b72, r9=r9, g728=g728, h8128=h8128, eye=eye)


def _pad_x(x16):
    """(n, 64, 192, 192) fp16 -> (n, 128, 98*196): 96 rows split into two
    halves stacked in the partition dim, one reflected halo row above and
    below each half, and each row stored as [., padL, 192 cols, padR, .]
    so 3x3 taps on device are plain shifted flat views."""
    n = x16.shape[0]
    xp = np.zeros((n, 64, 2, NROWS, RS), dtype=np.float16)
    xp[:, :, :, 1:97, 2:194] = x16.reshape(n, 64, 2, RH, W)
    xp[:, :, 0, 0, 2:194] = x16[:, :, 1]        # reflect of row -1
    xp[:, :, 1, 0, 2:194] = x16[:, :, 95]       # halo above bottom half
    xp[:, :, 0, 97, 2:194] = x16[:, :, 96]      # halo below top half
    xp[:, :, 1, 97, 2:194] = x16[:, :, 190]     # reflect of row 192
    xp[..., 1] = xp[..., 3]                     # reflect of col -1
    xp[..., 194] = xp[..., 192]                 # reflect of col 192
    return np.ascontiguousarray(xp.reshape(n, 128, XPLEN))


def _prepare(x, conv_w, bn_gamma, bn_beta, bn_mean, bn_var):
    x16 = np.asarray(x, dtype=np.float16)
    xp = _pad_x(x16)
    consts = _host_consts(np.asarray(conv_w, np.float32),
                          np.asarray(bn_gamma, np.float32),
                          np.asarray(bn_beta, np.float32),
                          np.asarray(bn_mean, np.float32),
                          np.asarray(bn_var, np.float32))
    return [dict(x=xp[i], **consts) for i in range(N)]


def _unpad(a):
    """[128, 96*196] padded-rows -> (64, 192, 192) fp32."""
    return a.reshape(64, 2, RH, RS)[..., 2:194].reshape(
        64, H, W).astype(np.float32)


def _collect(res):
    low = np.stack([_unpad(res[i]["low"]) for i in range(N)])
    high = np.stack([_unpad(res[i]["high"]) for i in range(N)])
    return low, high


def kernel(x, conv_w, bn_gamma, bn_beta, bn_mean, bn_var):
    in_maps = _prepare(x, conv_w, bn_gamma, bn_beta, bn_mean, bn_var)
    nc = _get_program()
    res = run_bass_kernel_spmd(nc, in_maps, list(range(N))).results
    return _collect(res)


if __name__ == "__main__":
    rng = np.random.default_rng(0)
    demo = dict(
        x=rng.standard_normal((N, IC, H, W), dtype=np.float32),
        conv_w=rng.standard_normal((72, 64)).astype(np.float32),
        bn_gamma=np.ones(72, np.float32),
        bn_beta=np.zeros(72, np.float32),
        bn_mean=rng.standard_normal(72).astype(np.float32) * 0.1,
        bn_var=rng.uniform(0.5, 1.5, 72).astype(np.float32),
    )
    low, high = kernel(**demo)
    print("ok", low.shape, high.shape)
